# revision 24
# baseline (speedup 1.0000x reference)
"""Single-head attention (B=4, N=2048, D=1024) on 8 Trainium2 NeuronCores.

Sharding: data-parallel over (batch, query-half): core c handles batch c//2,
query rows (c%2)*1024 ... +1024.  Each core computes the full K/V projections
for its batch (duplicated within each pair of cores sharing a batch; avoids
any cross-core communication), its own Q projection, unnormalized
exp(q.k/sqrt(D)) in transposed [key, query] layout, and A@V plus the softmax
denominator via a ones-column matmul.  All matmuls bf16 with fp32 PSUM
accumulation; softmax math (exp/recip/scale) in fp32.

The host pre-transposes x and the weights, casts to bf16, and rotates key
columns so each core's queries sit at columns 0:1024 of its xT (attention is
permutation-invariant over keys, so key order doesn't matter).
"""

from contextlib import ExitStack

import ml_dtypes
import numpy as np

import concourse.bass as bass
import concourse.mybir as mybir
import concourse.tile as tile
from concourse.bass_utils import run_bass_kernel_spmd

B, N, D = 4, 2048, 1024
NCORES = 8
P = 128
NQ = N // 2          # queries per core
NK = N               # keys per core
DC = D // P          # 8 contraction chunks for projections
EC = D // P          # 8 embed blocks
JB = NK // P         # 16 key blocks
F = 512              # matmul moving free dim (one PSUM bank of fp32)
SCALE = 1.0 / np.sqrt(D)

BF = mybir.dt.bfloat16
F32 = mybir.dt.float32


def _attention_kernel(ctx, tc, out, xT, wqT, wkT, wvT):
    nc = tc.nc

    consts = ctx.enter_context(tc.tile_pool(name="consts", bufs=1))
    psmain = ctx.enter_context(tc.tile_pool(name="psmain", bufs=2, space="PSUM"))
    psav = ctx.enter_context(tc.tile_pool(name="psav", bufs=6, space="PSUM"))
    outp = ctx.enter_context(tc.tile_pool(name="outp", bufs=2))
    small = ctx.enter_context(tc.tile_pool(name="small", bufs=2))

    # Resident SBUF tensors (SBUF is 192KB/partition; these sum to ~176KB).
    # wv and qT share one buffer: wv's last read (phase 1b) precedes qT's
    # first write (phase 2a) on the serial PE/DVE streams.
    xT_sb = consts.tile([P, DC, NK], BF, tag="xT")      # [p, d-chunk, key]
    wq_sb = consts.tile([P, DC, D], BF, tag="wq")
    wk_sb = consts.tile([P, DC, D], BF, tag="wk")
    wv_sb = consts.tile([P, DC, D], BF, tag="wvq")      # phase 1: WvT
    qT_sb = wv_sb                                        # phase 2: qT[e, i]
    kT_sb = consts.tile([P, EC, NK], BF, tag="kT")      # [p, e-block, key]
    v_sb = consts.tile([P, JB, D], BF, tag="v")         # [p, key-block, e]
    pT_sb = consts.tile([P, JB, NQ], BF, tag="pT")      # [p, key-block, query]
    ones_sb = consts.tile([P, 1], BF, tag="ones")

    nc.vector.memset(ones_sb, 1.0)

    xTr = xT.rearrange("(c p) j -> p c j", p=P)
    wqr = wqT.rearrange("(c p) e -> p c e", p=P)
    wkr = wkT.rearrange("(c p) e -> p c e", p=P)
    wvr = wvT.rearrange("(c p) e -> p c e", p=P)
    # Inputs on the HW DGE (sync) for low-latency loads; outputs go out via
    # the SW DGE (gpsimd) because a HWDGE ring that carried loads forces a
    # queue-drain wait onto a later store (direction transition), which
    # would give the store two sync waits (HW allows one).
    wv_dmas = []
    in_dmas = []
    for c in range(DC):
        in_dmas.append(nc.sync.dma_start(out=wk_sb[:, c, :], in_=wkr[:, c, :]))
        in_dmas.append(nc.sync.dma_start(out=xT_sb[:, c, :], in_=xTr[:, c, :]))
        wv_dmas.append(nc.sync.dma_start(out=wv_sb[:, c, :], in_=wvr[:, c, :]))
        in_dmas.append(nc.sync.dma_start(out=wq_sb[:, c, :], in_=wqr[:, c, :]))
    in_dmas.extend(wv_dmas)

    def sp_observe(inst, why):
        # One-wait nops on the sync sequencer: make SP observe a proc's sem
        # tick so later SP instructions (out-DMAs, the kernel-tail drain)
        # don't need to aggregate multiple sync waits (HW allows one wait
        # per instruction; the auto-generated tail drain aggregates every
        # active proc otherwise).
        n = nc.sync.nop(hint="observe")
        tile.add_dep_helper(n.ins, inst.ins, reason=why)

    # Warm-up touches: attach each input tensor's chunk-0 DMA wait to a
    # dedicated trivial matmul while PSUM slots are fresh (slot reuse by a
    # PE-only group needs no cross-engine wait).  Without this, a phase's
    # first accumulation matmul can need both a DMA wait and a PSUM-WAR
    # wait, and PE matmuls support only a single sync-wait command
    # (walrus: "Too many sync wait commands").
    for t in (wk_sb, xT_sb, wv_sb, wq_sb):
        wm = psmain.tile([P, F], F32, tag="ps")
        nc.tensor.matmul(
            wm[0:1, 0:1], lhsT=t[:, 0, 0:1], rhs=t[:, 0, 0:1], start=True, stop=True
        )

    # Phase 1a: kT[e, j] — lhsT = WkT[d, e-blk], rhs = xT[d, j-tile]
    for e in range(EC):
        for jt in range(NK // F):
            ps = psmain.tile([P, F], F32, tag="ps")
            for c in range(DC):
                nc.tensor.matmul(
                    ps,
                    lhsT=wk_sb[:, c, e * P : (e + 1) * P],
                    rhs=xT_sb[:, c, jt * F : (jt + 1) * F],
                    start=(c == 0),
                    stop=(c == DC - 1),
                )
            nc.vector.tensor_copy(out=kT_sb[:, e, jt * F : (jt + 1) * F], in_=ps)

    # Phase 1b: v[j, e] — lhsT = xT[d, j-blk], rhs = WvT[d, e-tile]
    for j in range(JB):
        for et in range(D // F):
            ps = psmain.tile([P, F], F32, tag="ps")
            for c in range(DC):
                nc.tensor.matmul(
                    ps,
                    lhsT=xT_sb[:, c, j * P : (j + 1) * P],
                    rhs=wv_sb[:, c, et * F : (et + 1) * F],
                    start=(c == 0),
                    stop=(c == DC - 1),
                )
            nc.vector.tensor_copy(out=v_sb[:, j, et * F : (et + 1) * F], in_=ps)

    # DVE touches: qT overwrites wv's buffer, so the DVE must have observed
    # wv's input-DMA completion before its first qT write (WAW), or the qT
    # copy would need a DMA wait on top of its PE wait (one-wait limit).
    # The touches carry the DMA waits via explicit deps (reading wv itself
    # would add a DVE WAR self-wait on the later qT copies).  They run on
    # DVE after phase 1b's copies, when the DMAs are long done.
    # (self-copies: reading any DVE-written tensor would add a DVE self-wait
    # on top of the explicit DMA dep, and instructions get only one wait)
    touch = small.tile([P, DC], F32, tag="touch")
    for c in range(DC):
        t = nc.vector.tensor_copy(out=touch[0:1, c : c + 1], in_=touch[0:1, c : c + 1])
        tile.add_dep_helper(t.ins, wv_dmas[c].ins, reason="observe wv DMA on DVE")

    # Phase 2a: qT[e, i] — queries are xT columns 0:NQ (host rotates keys)
    for e in range(EC):
        for it in range(NQ // F):
            ps = psmain.tile([P, F], F32, tag="ps")
            for c in range(DC):
                nc.tensor.matmul(
                    ps,
                    lhsT=wq_sb[:, c, e * P : (e + 1) * P],
                    rhs=xT_sb[:, c, it * F : (it + 1) * F],
                    start=(c == 0),
                    stop=(c == DC - 1),
                )
            nc.vector.tensor_copy(out=qT_sb[:, e, it * F : (it + 1) * F], in_=ps)

    # Phase 2b: scoresT[j, i] = k @ q.T, then p = exp(scores * SCALE)
    for j in range(JB):
        for it in range(NQ // F):
            ps = psmain.tile([P, F], F32, tag="ps")
            for e in range(EC):
                nc.tensor.matmul(
                    ps,
                    lhsT=kT_sb[:, e, j * P : (j + 1) * P],
                    rhs=qT_sb[:, e, it * F : (it + 1) * F],
                    start=(e == 0),
                    stop=(e == EC - 1),
                )
            last_exp = nc.scalar.activation(
                out=pT_sb[:, j, it * F : (it + 1) * F],
                in_=ps,
                func=mybir.ActivationFunctionType.Exp,
                scale=float(SCALE),
            )

    # Let SP observe all input-DMA completions so the out-DMAs below carry
    # only their data-ready wait (not a DMA-queue-order wait on top).
    for d in in_dmas:
        sp_observe(d, "observe input DMA on SP")

    # Phase 2c: out[i, e] = (pT.T @ v) / (pT.T @ ones)
    oguard = small.tile([P, NQ // P], F32, tag="oguard")
    out_dmas = []
    for ib in range(NQ // P):
        po0 = psav.tile([P, F], F32, tag="po")
        po1 = psav.tile([P, F], F32, tag="po")
        pd = psav.tile([P, F], F32, tag="po")
        for j in range(JB):
            lhsT = pT_sb[:, j, ib * P : (ib + 1) * P]
            nc.tensor.matmul(
                po0, lhsT=lhsT, rhs=v_sb[:, j, 0:F], start=(j == 0), stop=(j == JB - 1)
            )
            nc.tensor.matmul(
                po1, lhsT=lhsT, rhs=v_sb[:, j, F : 2 * F],
                start=(j == 0), stop=(j == JB - 1),
            )
            last_mm = nc.tensor.matmul(
                pd[:, 0:1], lhsT=lhsT, rhs=ones_sb, start=(j == 0), stop=(j == JB - 1)
            )
        o_sb = outp.tile([P, D], F32, tag="o")
        if ib >= 2:
            # Pre-observe the output-DMA tick (WAR on o_sb slot reuse) on
            # the DVE via a guard copy to a once-written scratch slice, so
            # the tensor_scalars below carry only the recip self-wait
            # (one-wait limit per DVE instruction).
            g = nc.vector.tensor_copy(
                out=oguard[0:1, ib : ib + 1], in_=oguard[0:1, ib : ib + 1]
            )
            tile.add_dep_helper(
                g.ins, out_dmas[ib - 2].ins, reason="observe out DMA on DVE"
            )
        recip = small.tile([P, 1], F32, tag="recip")
        nc.vector.reciprocal(recip, pd[:, 0:1])
        nc.vector.tensor_scalar_mul(o_sb[:, 0:F], po0, recip)
        last_ts = nc.vector.tensor_scalar_mul(o_sb[:, F : 2 * F], po1, recip)
        out_dmas.append(
            nc.gpsimd.dma_start(out=out[ib * P : (ib + 1) * P, :], in_=o_sb)
        )

    # Let SP observe every remaining proc's final tick so the auto-generated
    # kernel-tail drain needs no aggregated multi-sem wait of its own.
    for d in out_dmas:
        sp_observe(d, "observe output DMA on SP")
    sp_observe(last_exp, "observe ACT on SP")
    sp_observe(last_mm, "observe PE on SP")
    sp_observe(last_ts, "observe DVE on SP")


def build_attention_module():
    nc = bass.Bass(trn_type="TRN2", target_bir_lowering=False, debug=False)
    xT = nc.dram_tensor("xT", [D, NK], BF, kind="ExternalInput").ap()
    wqT = nc.dram_tensor("wqT", [D, D], BF, kind="ExternalInput").ap()
    wkT = nc.dram_tensor("wkT", [D, D], BF, kind="ExternalInput").ap()
    wvT = nc.dram_tensor("wvT", [D, D], BF, kind="ExternalInput").ap()
    out = nc.dram_tensor("out", [NQ, D], F32, kind="ExternalOutput").ap()
    with tile.TileContext(nc) as tc:
        with ExitStack() as ctx:
            _attention_kernel(ctx, tc, out, xT, wqT, wkT, wvT)
    return nc


_module_cache = None


def _get_module():
    global _module_cache
    if _module_cache is None:
        _module_cache = build_attention_module()
    return _module_cache


def make_in_maps(x, Wq, Wk, Wv):
    bf = ml_dtypes.bfloat16
    x = np.asarray(x, dtype=np.float32)
    wq = np.asarray(Wq, dtype=np.float32).T.astype(bf)
    wk = np.asarray(Wk, dtype=np.float32).T.astype(bf)
    wv = np.asarray(Wv, dtype=np.float32).T.astype(bf)
    in_maps = []
    for core in range(NCORES):
        b, half = divmod(core, 2)
        xt = x[b].T  # [D, N]
        if half == 1:
            xt = np.roll(xt, -NQ, axis=1)  # queries to columns 0:NQ
        in_maps.append(
            {"xT": xt.astype(bf), "wqT": wq, "wkT": wk, "wvT": wv}
        )
    return in_maps


def _install_ntff_hook_shim():
    """The container's `antenv` stub lacks axon_hooks; register an equivalent
    built on trn_agent_boot's ctypes NTFF driver so trace=True works."""
    import sys
    import types

    if "antenv.axon_hooks" in sys.modules:
        return
    try:
        from trn_agent_boot.trn_boot import _ntff_profile_via_ctypes

        hook = _ntff_profile_via_ctypes("/opt/axon/libaxon_pjrt.so")
    except Exception:
        hook = None
    mod = types.ModuleType("antenv.axon_hooks")
    mod.get_axon_ntff_profile_hook = lambda: hook
    sys.modules["antenv.axon_hooks"] = mod


def kernel(x, Wq, Wk, Wv, _trace=False, _trace_cores=None):
    if _trace:
        _install_ntff_hook_shim()
    in_maps = make_in_maps(x, Wq, Wk, Wv)
    nc = _get_module()
    res = run_bass_kernel_spmd(
        nc,
        in_maps,
        core_ids=list(range(NCORES)),
        trace=_trace,
        trace_cores=_trace_cores,
    )
    out = np.empty((B, N, D), dtype=np.float32)
    for core in range(NCORES):
        b, half = divmod(core, 2)
        out[b, half * NQ : (half + 1) * NQ] = res.results[core]["out"]
    if _trace:
        return out, res
    return out


# revision 26
# speedup vs baseline: 1.1443x; 1.1443x over previous
"""Single-head attention (B=4, N=2048, D=1024) on 8 Trainium2 NeuronCores.

Sharding: data-parallel over (batch, query-half): core c handles batch c//2,
query rows (c%2)*1024 ... +1024.  Each core computes the full K/V projections
for its batch (duplicated within each pair of cores sharing a batch; avoids
any cross-core communication), its own Q projection, unnormalized
exp(q.k/sqrt(D)) in transposed [key, query] layout, and A@V plus the softmax
denominator via a ones-column matmul.  All matmuls bf16 with fp32 PSUM
accumulation; softmax math (exp/recip/scale) in fp32.

The host pre-transposes x and the weights, casts to bf16, and rotates key
columns so each core's queries sit at columns 0:1024 of its xT (attention is
permutation-invariant over keys, so key order doesn't matter).
"""

from contextlib import ExitStack

import ml_dtypes
import numpy as np

import concourse.bass as bass
import concourse.mybir as mybir
import concourse.tile as tile
from concourse.bass_utils import run_bass_kernel_spmd

B, N, D = 4, 2048, 1024
NCORES = 8
P = 128
NQ = N // 2          # queries per core
NK = N               # keys per core
DC = D // P          # 8 contraction chunks for projections
EC = D // P          # 8 embed blocks
JB = NK // P         # 16 key blocks
F = 512              # matmul moving free dim (one PSUM bank of fp32)
SCALE = 1.0 / np.sqrt(D)

BF = mybir.dt.bfloat16
F32 = mybir.dt.float32


def _attention_kernel(ctx, tc, out, xT, wqT, wkT, wvT):
    nc = tc.nc

    consts = ctx.enter_context(tc.tile_pool(name="consts", bufs=1))
    psmain = ctx.enter_context(tc.tile_pool(name="psmain", bufs=2, space="PSUM"))
    psav = ctx.enter_context(tc.tile_pool(name="psav", bufs=6, space="PSUM"))
    outp = ctx.enter_context(tc.tile_pool(name="outp", bufs=2))
    small = ctx.enter_context(tc.tile_pool(name="small", bufs=2))

    # Resident SBUF tensors (SBUF is 192KB/partition; these sum to ~176KB).
    # wv and qT share one buffer: wv's last read (phase 1b) precedes qT's
    # first write (phase 2a) on the serial PE/DVE streams.
    xT_sb = consts.tile([P, DC, NK], BF, tag="xT")      # [p, d-chunk, key]
    wq_sb = consts.tile([P, DC, D], BF, tag="wq")
    wk_sb = consts.tile([P, DC, D], BF, tag="wk")
    wv_sb = consts.tile([P, DC, D], BF, tag="wvq")      # phase 1: WvT
    qT_sb = wv_sb                                        # phase 2: qT[e, i]
    kT_sb = consts.tile([P, EC, NK], BF, tag="kT")      # [p, e-block, key]
    v_sb = consts.tile([P, JB, D], BF, tag="v")         # [p, key-block, e]
    pT_sb = consts.tile([P, JB, NQ], BF, tag="pT")      # [p, key-block, query]
    ones_sb = consts.tile([P, 1], BF, tag="ones")

    nc.vector.memset(ones_sb, 1.0)

    xTr = xT.rearrange("(c p) j -> p c j", p=P)
    wqr = wqT.rearrange("(c p) e -> p c e", p=P)
    wkr = wkT.rearrange("(c p) e -> p c e", p=P)
    wvr = wvT.rearrange("(c p) e -> p c e", p=P)
    # Inputs on the HW DGE (sync) for low-latency loads; outputs go out via
    # the SW DGE (gpsimd) because a HWDGE ring that carried loads forces a
    # queue-drain wait onto a later store (direction transition), which
    # would give the store two sync waits (HW allows one).
    # Issue order follows consumption order: phase 1a drains all of wk and
    # xT within its first few accumulation groups; wv/wq aren't needed
    # until phases 1b/2a and shouldn't compete for HBM bandwidth early.
    wv_dmas = []
    in_dmas = []
    for c in range(DC):
        in_dmas.append(nc.sync.dma_start(out=wk_sb[:, c, :], in_=wkr[:, c, :]))
        in_dmas.append(nc.sync.dma_start(out=xT_sb[:, c, :], in_=xTr[:, c, :]))
    for c in range(DC):
        wv_dmas.append(nc.sync.dma_start(out=wv_sb[:, c, :], in_=wvr[:, c, :]))
    for c in range(DC):
        in_dmas.append(nc.sync.dma_start(out=wq_sb[:, c, :], in_=wqr[:, c, :]))
    in_dmas.extend(wv_dmas)

    def sp_observe(inst, why):
        # One-wait nops on the sync sequencer: make SP observe a proc's sem
        # tick so later SP instructions (out-DMAs, the kernel-tail drain)
        # don't need to aggregate multiple sync waits (HW allows one wait
        # per instruction; the auto-generated tail drain aggregates every
        # active proc otherwise).
        n = nc.sync.nop(hint="observe")
        tile.add_dep_helper(n.ins, inst.ins, reason=why)

    # HAM pre-warm: the PE clock-gate opens only after ~3.4us of sustained
    # matmul activity, and the first real matmuls can't start until the
    # first input chunks land (~5-10us of DMA).  Burn that idle time on
    # dummy matmuls over zeroed SBUF so the real work runs at 2.4GHz from
    # its first instruction instead of 1.2GHz for its first ~30us.
    warm_src = small.tile([P, 640], BF, tag="warm")
    nc.vector.memset(warm_src, 0.0)
    warm_ps = psmain.tile([P, F], F32, tag="ps")
    N_WARM = 14
    for w in range(N_WARM):
        nc.tensor.matmul(
            warm_ps,
            lhsT=warm_src[:, 0:P],
            rhs=warm_src[:, P : P + F],
            start=(w == 0),
            stop=(w == N_WARM - 1),
        )

    # Warm-up touches: attach each input tensor's chunk-0 DMA wait to a
    # dedicated trivial matmul while PSUM slots are fresh (slot reuse by a
    # PE-only group needs no cross-engine wait).  Without this, a phase's
    # first accumulation matmul can need both a DMA wait and a PSUM-WAR
    # wait, and PE matmuls support only a single sync-wait command
    # (walrus: "Too many sync wait commands").
    for t in (wk_sb, xT_sb, wv_sb, wq_sb):
        wm = psmain.tile([P, F], F32, tag="ps")
        nc.tensor.matmul(
            wm[0:1, 0:1], lhsT=t[:, 0, 0:1], rhs=t[:, 0, 0:1], start=True, stop=True
        )

    # Phase 1a: kT[e, j] — lhsT = WkT[d, e-blk], rhs = xT[d, j-tile]
    for e in range(EC):
        for jt in range(NK // F):
            ps = psmain.tile([P, F], F32, tag="ps")
            for c in range(DC):
                nc.tensor.matmul(
                    ps,
                    lhsT=wk_sb[:, c, e * P : (e + 1) * P],
                    rhs=xT_sb[:, c, jt * F : (jt + 1) * F],
                    start=(c == 0),
                    stop=(c == DC - 1),
                )
            nc.vector.tensor_copy(out=kT_sb[:, e, jt * F : (jt + 1) * F], in_=ps)

    # Phase 1b: v[j, e] — lhsT = xT[d, j-blk], rhs = WvT[d, e-tile]
    for j in range(JB):
        for et in range(D // F):
            ps = psmain.tile([P, F], F32, tag="ps")
            for c in range(DC):
                nc.tensor.matmul(
                    ps,
                    lhsT=xT_sb[:, c, j * P : (j + 1) * P],
                    rhs=wv_sb[:, c, et * F : (et + 1) * F],
                    start=(c == 0),
                    stop=(c == DC - 1),
                )
            nc.vector.tensor_copy(out=v_sb[:, j, et * F : (et + 1) * F], in_=ps)

    # DVE touches: qT overwrites wv's buffer, so the DVE must have observed
    # wv's input-DMA completion before its first qT write (WAW), or the qT
    # copy would need a DMA wait on top of its PE wait (one-wait limit).
    # The touches carry the DMA waits via explicit deps (reading wv itself
    # would add a DVE WAR self-wait on the later qT copies).  They run on
    # DVE after phase 1b's copies, when the DMAs are long done.
    # (self-copies: reading any DVE-written tensor would add a DVE self-wait
    # on top of the explicit DMA dep, and instructions get only one wait)
    touch = small.tile([P, DC], F32, tag="touch")
    for c in range(DC):
        t = nc.vector.tensor_copy(out=touch[0:1, c : c + 1], in_=touch[0:1, c : c + 1])
        tile.add_dep_helper(t.ins, wv_dmas[c].ins, reason="observe wv DMA on DVE")

    # Phase 2a: qT[e, i] — queries are xT columns 0:NQ (host rotates keys)
    for e in range(EC):
        for it in range(NQ // F):
            ps = psmain.tile([P, F], F32, tag="ps")
            for c in range(DC):
                nc.tensor.matmul(
                    ps,
                    lhsT=wq_sb[:, c, e * P : (e + 1) * P],
                    rhs=xT_sb[:, c, it * F : (it + 1) * F],
                    start=(c == 0),
                    stop=(c == DC - 1),
                )
            nc.vector.tensor_copy(out=qT_sb[:, e, it * F : (it + 1) * F], in_=ps)

    # Phase 2b: scoresT[j, i] = k @ q.T, then p = exp(scores * SCALE)
    for j in range(JB):
        for it in range(NQ // F):
            ps = psmain.tile([P, F], F32, tag="ps")
            for e in range(EC):
                nc.tensor.matmul(
                    ps,
                    lhsT=kT_sb[:, e, j * P : (j + 1) * P],
                    rhs=qT_sb[:, e, it * F : (it + 1) * F],
                    start=(e == 0),
                    stop=(e == EC - 1),
                )
            last_exp = nc.scalar.activation(
                out=pT_sb[:, j, it * F : (it + 1) * F],
                in_=ps,
                func=mybir.ActivationFunctionType.Exp,
                scale=float(SCALE),
            )

    # Let SP observe all input-DMA completions so the out-DMAs below carry
    # only their data-ready wait (not a DMA-queue-order wait on top).
    for d in in_dmas:
        sp_observe(d, "observe input DMA on SP")

    # Phase 2c: out[i, e] = (pT.T @ v) / (pT.T @ ones)
    oguard = small.tile([P, NQ // P], F32, tag="oguard")
    out_dmas = []
    for ib in range(NQ // P):
        po0 = psav.tile([P, F], F32, tag="po")
        po1 = psav.tile([P, F], F32, tag="po")
        pd = psav.tile([P, F], F32, tag="po")
        for j in range(JB):
            lhsT = pT_sb[:, j, ib * P : (ib + 1) * P]
            nc.tensor.matmul(
                po0, lhsT=lhsT, rhs=v_sb[:, j, 0:F], start=(j == 0), stop=(j == JB - 1)
            )
            nc.tensor.matmul(
                po1, lhsT=lhsT, rhs=v_sb[:, j, F : 2 * F],
                start=(j == 0), stop=(j == JB - 1),
            )
            last_mm = nc.tensor.matmul(
                pd[:, 0:1], lhsT=lhsT, rhs=ones_sb, start=(j == 0), stop=(j == JB - 1)
            )
        o_sb = outp.tile([P, D], F32, tag="o")
        if ib >= 2:
            # Pre-observe the output-DMA tick (WAR on o_sb slot reuse) on
            # the DVE via a guard copy to a once-written scratch slice, so
            # the tensor_scalars below carry only the recip self-wait
            # (one-wait limit per DVE instruction).
            g = nc.vector.tensor_copy(
                out=oguard[0:1, ib : ib + 1], in_=oguard[0:1, ib : ib + 1]
            )
            tile.add_dep_helper(
                g.ins, out_dmas[ib - 2].ins, reason="observe out DMA on DVE"
            )
        recip = small.tile([P, 1], F32, tag="recip")
        nc.vector.reciprocal(recip, pd[:, 0:1])
        nc.vector.tensor_scalar_mul(o_sb[:, 0:F], po0, recip)
        last_ts = nc.vector.tensor_scalar_mul(o_sb[:, F : 2 * F], po1, recip)
        out_dmas.append(
            nc.gpsimd.dma_start(out=out[ib * P : (ib + 1) * P, :], in_=o_sb)
        )

    # Let SP observe every remaining proc's final tick so the auto-generated
    # kernel-tail drain needs no aggregated multi-sem wait of its own.
    for d in out_dmas:
        sp_observe(d, "observe output DMA on SP")
    sp_observe(last_exp, "observe ACT on SP")
    sp_observe(last_mm, "observe PE on SP")
    sp_observe(last_ts, "observe DVE on SP")


def build_attention_module():
    nc = bass.Bass(trn_type="TRN2", target_bir_lowering=False, debug=False)
    xT = nc.dram_tensor("xT", [D, NK], BF, kind="ExternalInput").ap()
    wqT = nc.dram_tensor("wqT", [D, D], BF, kind="ExternalInput").ap()
    wkT = nc.dram_tensor("wkT", [D, D], BF, kind="ExternalInput").ap()
    wvT = nc.dram_tensor("wvT", [D, D], BF, kind="ExternalInput").ap()
    out = nc.dram_tensor("out", [NQ, D], F32, kind="ExternalOutput").ap()
    with tile.TileContext(nc) as tc:
        with ExitStack() as ctx:
            _attention_kernel(ctx, tc, out, xT, wqT, wkT, wvT)
    return nc


_module_cache = None


def _get_module():
    global _module_cache
    if _module_cache is None:
        _module_cache = build_attention_module()
    return _module_cache


def make_in_maps(x, Wq, Wk, Wv):
    bf = ml_dtypes.bfloat16
    x = np.asarray(x, dtype=np.float32)
    wq = np.asarray(Wq, dtype=np.float32).T.astype(bf)
    wk = np.asarray(Wk, dtype=np.float32).T.astype(bf)
    wv = np.asarray(Wv, dtype=np.float32).T.astype(bf)
    in_maps = []
    for core in range(NCORES):
        b, half = divmod(core, 2)
        xt = x[b].T  # [D, N]
        if half == 1:
            xt = np.roll(xt, -NQ, axis=1)  # queries to columns 0:NQ
        in_maps.append(
            {"xT": xt.astype(bf), "wqT": wq, "wkT": wk, "wvT": wv}
        )
    return in_maps


def _install_ntff_hook_shim():
    """The container's `antenv` stub lacks axon_hooks; register an equivalent
    built on trn_agent_boot's ctypes NTFF driver so trace=True works."""
    import sys
    import types

    if "antenv.axon_hooks" in sys.modules:
        return
    try:
        from trn_agent_boot.trn_boot import _ntff_profile_via_ctypes

        hook = _ntff_profile_via_ctypes("/opt/axon/libaxon_pjrt.so")
    except Exception:
        hook = None
    mod = types.ModuleType("antenv.axon_hooks")
    mod.get_axon_ntff_profile_hook = lambda: hook
    sys.modules["antenv.axon_hooks"] = mod


def kernel(x, Wq, Wk, Wv, _trace=False, _trace_cores=None):
    if _trace:
        _install_ntff_hook_shim()
    in_maps = make_in_maps(x, Wq, Wk, Wv)
    nc = _get_module()
    res = run_bass_kernel_spmd(
        nc,
        in_maps,
        core_ids=list(range(NCORES)),
        trace=_trace,
        trace_cores=_trace_cores,
    )
    out = np.empty((B, N, D), dtype=np.float32)
    for core in range(NCORES):
        b, half = divmod(core, 2)
        out[b, half * NQ : (half + 1) * NQ] = res.results[core]["out"]
    if _trace:
        return out, res
    return out
